# revision 21
# baseline (speedup 1.0000x reference)
"""DiffPool (dense_diff_pool + DenseSAGEConv) on 8 Trainium2 NeuronCores.

Sharding: data-parallel over the graph batch dimension B. Each core owns
B/8 = 16 graphs: it densifies its own [16, N, N] adjacency on-device from
per-row CSR index lists (gpsimd local_scatter), runs the SAGE conv +
softmax assignment + pooling bmms per graph entirely out of SBUF, and
emits per-core partials for the scalar link/entropy losses, which the
host combines (equivalent to the all-reduce of two scalars).

Numerics: the adjacency is built in fp16 (integer edge counts, exact);
matmul operands are fp16 with fp32 PSUM accumulation; softmax and loss
reductions run in fp32. The entropy uses the exact softmax identity
ln(softmax) = (logit - max) - ln(sum_exp), so no per-row Ln is needed;
||A||_F^2 is a sum of squared integer edge counts, computed on the host
from the same CSR counts the device scatters.

Host-side work is limited to index arithmetic / layout prep: intra-graph
node positions, per-(graph,node) CSR lists with duplicate-edge counts,
transposing x, and final scalar-loss assembly.
"""

import sys

for _p in ("/opt/trn_rl_repo",):
    if _p not in sys.path:
        sys.path.append(_p)

import numpy as np
import ml_dtypes

import concourse.bass as bass
import concourse.bacc as bacc
import concourse.mybir as mybir
import concourse.tile as tile
from concourse.masks import make_identity
from concourse.bass_utils import run_bass_kernel_spmd

F32 = mybir.dt.float32
F16 = mybir.dt.float16
I16 = mybir.dt.int16
AF = mybir.ActivationFunctionType
ALU = mybir.AluOpType
AX = mybir.AxisListType

B = 128      # graphs in batch
N = 512      # max nodes per graph
D = 128      # node features
C = 64       # clusters
P = 128      # SBUF partitions
NCORES = 8
G = B // NCORES          # graphs per core
TPG = N // P             # 128-row tiles per graph
NT = G * TPG             # v-tiles per core
NT2 = NT // 2            # packed (2-tile) scatter calls per core
EPS = 1e-15
YW = 2 * C + 1           # per-nb y layout: [ones(1) | rel(64) | root(64)]

_KERNEL_CACHE = {}


def _build(NI2, has_bias):
    nc = bacc.Bacc("TRN2", target_bir_lowering=False, debug=False, num_devices=NCORES)

    lsidx = nc.declare_dram_parameter("lsidx", [NT2, P, NI2], I16, isOutput=False)
    lsval = nc.declare_dram_parameter("lsval", [NT2, P, NI2], F16, isOutput=False)
    xt_in = nc.declare_dram_parameter("xt", [G, D, N], F16, isOutput=False)
    xr_in = nc.declare_dram_parameter("xr", [G, N, D], F16, isOutput=False)
    wb_in = nc.declare_dram_parameter("wb", [D + 1, 2 * C], F16, isOutput=False)
    mk_in = nc.declare_dram_parameter("mask", [NT, P], F32, isOutput=False)

    oxt_out = nc.declare_dram_parameter("oxt", [D, G * C], F32, isOutput=True)
    oadj_out = nc.declare_dram_parameter("oadj", [C, G * 2 * C], F32, isOutput=True)
    acc3_out = nc.declare_dram_parameter("acc3", [P, 4], F32, isOutput=True)

    with tile.TileContext(nc) as tc:
        with (
            tc.tile_pool(name="const", bufs=1) as cp,
            tc.tile_pool(name="work", bufs=2) as wp,
            tc.tile_pool(name="work3", bufs=3) as wp3,
            tc.tile_pool(name="work4", bufs=4) as wp4,
            tc.tile_pool(name="ps1", bufs=1, space="PSUM") as pp1,
            tc.tile_pool(name="ps2", bufs=2, space="PSUM") as pp2,
            tc.tile_pool(name="ps3", bufs=3, space="PSUM") as pp3,
        ):
            # ---------- preload ----------
            idx_sb = cp.tile([P, NT2 * NI2], I16)
            nc.sync.dma_start(out=idx_sb[:], in_=lsidx.ap().rearrange("t p i -> p t i"))
            val_sb = cp.tile([P, NT2 * NI2], F16)
            nc.sync.dma_start(out=val_sb[:], in_=lsval.ap().rearrange("t p i -> p t i"))
            wb_sb = cp.tile([P, 2 * C], F16)
            nc.scalar.dma_start(out=wb_sb[:], in_=wb_in.ap()[0:D])
            bias_sb = cp.tile([1, 2 * C], F16)
            nc.scalar.dma_start(out=bias_sb[:], in_=wb_in.ap()[D:D + 1])
            xt_sb = cp.tile([P, G * N], F16)
            nc.scalar.dma_start(out=xt_sb[:], in_=xt_in.ap().rearrange("g d n -> d g n"))
            mask_sb = cp.tile([P, NT], F32)
            nc.scalar.dma_start(out=mask_sb[:], in_=mk_in.ap().rearrange("t p -> p t"))
            xr_sb = cp.tile([P, NT * D], F16)
            nc.sync.dma_start(
                out=xr_sb[:], in_=xr_in.ap().rearrange("g (t p) d -> p (g t) d", p=P)
            )

            ones_k1 = cp.tile([1, P], F16)
            nc.vector.memset(ones_k1[:], 1.0)
            warm_sb = cp.tile([P, N], F16)
            nc.vector.memset(warm_sb[:], 0.0)
            for _w in range(40):
                warm_ps = pp3.tile([P, N], F32, tag="r_ps", name=f"warm_{_w}")
                nc.tensor.matmul(out=warm_ps[:], lhsT=warm_sb[:, 0:P],
                                 rhs=warm_sb[:], start=True, stop=True)
            acc3 = cp.tile([P, 4], F32)
            nc.vector.memset(acc3[:], 0.0)
            oxt_sb = cp.tile([D, G * C], F32)
            oadj_sb = cp.tile([C, G * 2 * C], F32)

            # per-(graph, ub) column stats, one column per u-tile
            NM_all = cp.tile([P, NT], F32)   # negated row max of logits
            RS_all = cp.tile([P, NT], F32)   # sum exp
            RI_all = cp.tile([P, NT], F32)   # 1 / sum exp
            ES_all = cp.tile([P, NT], F32)   # sum_c e * slog
            fin_a = cp.tile([P, NT], F32)
            fin_b = cp.tile([P, NT], F32)

            for g in range(G):
                g4 = g * TPG
                # ---------- densify adjacency (adjT, v-partitioned, packed) ----------
                adjp = [wp4.tile([P, 2 * N], F16, tag=f"adjp{h}", name=f"adjp{h}_{g}")
                        for h in range(2)]
                for h in range(2):
                    t2 = g * 2 + h
                    nc.gpsimd.local_scatter(
                        out_ap=adjp[h][:],
                        data_ap=val_sb[:, t2 * NI2:(t2 + 1) * NI2],
                        idxs_ap=idx_sb[:, t2 * NI2:(t2 + 1) * NI2],
                        channels=P, num_elems=2 * N, num_idxs=NI2,
                    )

                def adj_lhsT(vb, ub):
                    off = (vb % 2) * N + ub * P
                    return adjp[vb // 2][:, off:off + P]

                # ---------- y = [1 | x@W_rel^T | x@W_root^T + b_root] ----------
                y_sb = wp3.tile([P, TPG * YW], F16, tag="y")
                nc.vector.memset(
                    y_sb[:].rearrange("p (n w) -> p n w", w=YW)[:, :, 0:1], 1.0)
                y_ps = pp1.tile([P, TPG * 2 * C], F32, tag="y_ps")
                for nb in range(TPG):
                    yo = nb * 2 * C
                    if has_bias:
                        nc.tensor.matmul(out=y_ps[:, yo:yo + 2 * C], lhsT=ones_k1[:],
                                         rhs=bias_sb[:], start=True, stop=False)
                    nc.tensor.matmul(out=y_ps[:, yo:yo + 2 * C],
                                     lhsT=xt_sb[:, g * N + nb * P: g * N + (nb + 1) * P],
                                     rhs=wb_sb[:], start=not has_bias, stop=True)
                nc.any.tensor_copy(
                    out=y_sb[:].rearrange("p (n w) -> p n w", w=YW)[:, :, 1:YW],
                    in_=y_ps[:].rearrange("p (n w) -> p n w", w=2 * C))

                # ---------- z | deg (transposed wide form), logits ----------
                slog_all = wp3.tile([P, TPG * C], F32, tag="slog")
                e_all = wp3.tile([P, TPG * C], F32, tag="e_all")
                recip = wp.tile([P, TPG], F32, tag="recip")
                z_ps = pp2.tile([P, TPG * (C + 1)], F32, tag="z_ps")
                for ub in range(TPG):
                    zo = ub * (C + 1)
                    for vb in range(TPG):
                        nc.tensor.matmul(out=z_ps[:, zo:zo + C + 1],
                                         lhsT=adj_lhsT(vb, ub),
                                         rhs=y_sb[:, vb * YW: vb * YW + C + 1],
                                         start=(vb == 0), stop=(vb == TPG - 1))
                nc.vector.tensor_scalar_max(
                    recip[:, 0:TPG],
                    z_ps[:].rearrange("p (u w) -> p u w", w=C + 1)[:, :, 0:1], 1.0)
                nc.vector.reciprocal(recip[:, 0:TPG], recip[:, 0:TPG])
                for ub in range(TPG):
                    zo = ub * (C + 1)
                    # slog = z * (1/deg) + y_root
                    nc.vector.scalar_tensor_tensor(
                        out=slog_all[:, ub * C:(ub + 1) * C],
                        in0=z_ps[:, zo + 1:zo + C + 1], scalar=recip[:, ub:ub + 1],
                        in1=y_sb[:, ub * YW + C + 1:(ub + 1) * YW],
                        op0=ALU.mult, op1=ALU.add)

                # ---------- softmax ----------
                nc.vector.tensor_reduce(
                    out=NM_all[:, g4:g4 + TPG],
                    in_=slog_all[:].rearrange("p (u c) -> p u c", c=C),
                    axis=AX.X, op=ALU.max, negate=True)
                rs = [wp3.tile([P, 2 * C], F16, tag=f"rs{u}", name=f"rs{u}_{g}")
                      for u in range(TPG)]
                for ub in range(TPG):
                    nc.scalar.activation(
                        out=e_all[:, ub * C:(ub + 1) * C],
                        in_=slog_all[:, ub * C:(ub + 1) * C], func=AF.Exp,
                        bias=NM_all[:, g4 + ub:g4 + ub + 1], scale=1.0,
                        accum_out=RS_all[:, g4 + ub:g4 + ub + 1])
                nc.vector.reciprocal(RI_all[:, g4:g4 + TPG], RS_all[:, g4:g4 + TPG])
                for ub in range(TPG):
                    nc.any.tensor_scalar(
                        out=rs[ub][:, C:2 * C], in0=e_all[:, ub * C:(ub + 1) * C],
                        scalar1=RI_all[:, g4 + ub:g4 + ub + 1],
                        scalar2=mask_sb[:, g4 + ub:g4 + ub + 1],
                        op0=ALU.mult, op1=ALU.mult)

                # ---------- R = A @ sm ----------
                for ub in range(TPG):
                    r_ps = pp3.tile([P, C], F32, tag="r_ps")
                    for vb in range(TPG):
                        nc.tensor.matmul(out=r_ps[:], lhsT=adj_lhsT(vb, ub),
                                         rhs=rs[vb][:, C:2 * C],
                                         start=(vb == 0), stop=(vb == TPG - 1))
                    nc.any.tensor_copy(out=rs[ub][:, 0:C], in_=r_ps[:])

                # ---------- out_adj | G  and  out_x^T ----------
                og_ps_t = pp1.tile([C, 2 * C], F32, tag="og_ps")
                ox_ps_t = pp1.tile([D, C], F32, tag="ox_ps")
                og_ps = og_ps_t[:]
                ox_ps = ox_ps_t[:]
                for ub in range(TPG):
                    nc.tensor.matmul(out=og_ps, lhsT=rs[ub][:, C:2 * C],
                                     rhs=rs[ub][:], start=(ub == 0), stop=(ub == TPG - 1))
                    nc.tensor.matmul(out=ox_ps,
                                     lhsT=xr_sb[:, (g4 + ub) * D:(g4 + ub + 1) * D],
                                     rhs=rs[ub][:, C:2 * C],
                                     start=(ub == 0), stop=(ub == TPG - 1))
                nc.any.tensor_copy(out=oadj_sb[:, g * 2 * C:(g + 1) * 2 * C], in_=og_ps)
                nc.any.tensor_copy(out=oxt_sb[:, g * C:(g + 1) * C], in_=ox_ps)

                # ---------- entropy pieces: sum_c e * slog ----------
                eslog = wp3.tile([P, TPG * C], F32, tag="eslog")
                nc.vector.tensor_mul(out=eslog[:], in0=e_all[:], in1=slog_all[:])
                nc.vector.tensor_reduce(
                    out=ES_all[:, g4:g4 + TPG],
                    in_=eslog[:].rearrange("p (u c) -> p u c", c=C),
                    axis=AX.X, op=ALU.add)


            # ---------- entropy: sum_row mask * (ES*RI - (ln(RS) - NM)) ----------
            nc.scalar.activation(out=fin_a[:], in_=RS_all[:], func=AF.Ln)
            nc.vector.tensor_tensor(out=fin_a[:], in0=fin_a[:], in1=NM_all[:],
                                    op=ALU.subtract)
            nc.vector.tensor_mul(out=fin_b[:], in0=ES_all[:], in1=RI_all[:])
            nc.vector.tensor_tensor(out=fin_b[:], in0=fin_b[:], in1=fin_a[:],
                                    op=ALU.subtract)
            nc.vector.tensor_mul(out=fin_b[:], in0=fin_b[:], in1=mask_sb[:])
            nc.vector.tensor_reduce(out=acc3[:, 2:3], in_=fin_b[:], axis=AX.X,
                                    op=ALU.add)

            nc.sync.dma_start(out=oxt_out.ap(), in_=oxt_sb[:])
            nc.sync.dma_start(out=oadj_out.ap(), in_=oadj_sb[:])
            nc.sync.dma_start(out=acc3_out.ap(), in_=acc3[:])

    nc.finalize()
    return nc


def _get_nc(NI2, has_bias):
    key = (NI2, has_bias)
    if key not in _KERNEL_CACHE:
        _KERNEL_CACHE[key] = _build(NI2, has_bias)
    return _KERNEL_CACHE[key]


def _prep_inputs(x, edge_index, batch, W_rel, W_root, b_root):
    total = x.shape[0]
    counts = np.bincount(batch, minlength=B)
    ptr = np.zeros(B + 1, np.int64)
    np.cumsum(counts, out=ptr[1:])
    pos = np.arange(total, dtype=np.int64) - ptr[batch]

    if total == B * N and np.all(counts == N):
        dense_x = np.ascontiguousarray(x.reshape(B, N, D))
        mask = np.ones((B, N), np.float32)
    else:
        dense_x = np.zeros((B, N, D), np.float32)
        mask = np.zeros((B, N), np.float32)
        sel = pos < N
        dense_x[batch[sel], pos[sel]] = x[sel]
        mask[batch[sel], pos[sel]] = 1.0

    src, dst = edge_index[0].astype(np.int64), edge_index[1].astype(np.int64)
    b_e = batch[src].astype(np.int64)
    us, vs = pos[src], pos[dst]
    ok = (us < N) & (vs < N)
    key = ((b_e * N + vs) * N + us)[ok]  # adjT cell: row = g*N + v, col = u
    key.sort()
    if key.size:
        first = np.empty(key.size, bool)
        first[0] = True
        np.not_equal(key[1:], key[:-1], out=first[1:])
        ucell = key[first]
        starts = np.flatnonzero(first)
        cnt = np.diff(np.append(starts, key.size))
    else:
        ucell = key
        cnt = np.zeros(0, np.int64)
    ssqA = float((cnt.astype(np.float64) ** 2).sum())  # ||A||_F^2, exact

    rows = ucell // N                       # global g*N + v
    colu = ucell % N                        # u
    row_start = np.searchsorted(rows, np.arange(B * N))
    row_start_ext = np.append(row_start, ucell.size)
    rowcnt = np.diff(row_start_ext)         # unique-u count per adjT row
    pir = np.arange(ucell.size, dtype=np.int64) - row_start[rows]

    # packed scatter tiles: tile t2 = row // 256 covers 256 consecutive rows;
    # partition p = row % 128; rows with (row % 256) >= 128 go to cols +512.
    is_hi = (rows % 256) >= 128
    slot = pir + np.where(is_hi, rowcnt[np.maximum(rows - 128, 0)], 0)
    t2 = rows // 256
    pp = rows % 128
    colp = colu + np.where(is_hi, N, 0)
    max_slot = int(slot.max()) + 1 if ucell.size else 2
    NI2 = max(80, 2 * ((max_slot + 1) // 2))

    idx_np = np.full((B * 2, P, NI2), -1, np.int16)
    val_np = np.zeros((B * 2, P, NI2), np.float16)
    idx_np[t2, pp, slot] = colp.astype(np.int16)
    val_np[t2, pp, slot] = cnt

    wb = np.zeros((D + 1, 2 * C), np.float32)
    wb[0:D, 0:C] = W_rel.T
    wb[0:D, C:2 * C] = W_root.T
    wb[D, C:2 * C] = b_root

    xt = np.ascontiguousarray(
        dense_x.reshape(NCORES, G, N, D).transpose(0, 1, 3, 2)).astype(np.float16)
    xr = dense_x.reshape(NCORES, G, N, D).astype(np.float16)
    idx4 = idx_np.reshape(NCORES, NT2, P, NI2)
    val4 = val_np.reshape(NCORES, NT2, P, NI2)
    mk = mask.reshape(NCORES, NT, P)
    wb16 = wb.astype(np.float16)

    in_maps = []
    for c in range(NCORES):
        in_maps.append({
            "lsidx": idx4[c], "lsval": val4[c],
            "xt": xt[c], "xr": np.ascontiguousarray(xr[c]),
            "wb": wb16, "mask": np.ascontiguousarray(mk[c]),
        })
    return in_maps, NI2, ssqA


def kernel(x, edge_index, batch, W_rel, W_root, b_root):
    x = np.asarray(x, np.float32)
    edge_index = np.asarray(edge_index, np.int32)
    batch = np.asarray(batch, np.int32)
    W_rel = np.asarray(W_rel, np.float32)
    W_root = np.asarray(W_root, np.float32)
    b_root = np.asarray(b_root, np.float32)

    in_maps, NI2, ssqA = _prep_inputs(x, edge_index, batch, W_rel, W_root, b_root)
    nc = _get_nc(NI2, bool(np.any(b_root != 0.0)))
    res = run_bass_kernel_spmd(nc, in_maps, list(range(NCORES))).results

    oxt = np.stack([res[c]["oxt"] for c in range(NCORES)])    # [8, D, G*C]
    oadj = np.stack([res[c]["oadj"] for c in range(NCORES)])  # [8, C, G*2C]
    x_out = (oxt.reshape(NCORES, D, G, C)
             .transpose(0, 2, 3, 1).reshape(B * C, D).astype(np.float32))
    og = oadj.reshape(NCORES, C, G, 2 * C).transpose(0, 2, 1, 3).reshape(B, C, 2 * C)
    out_adj = np.ascontiguousarray(og[:, :, 0:C]).astype(np.float32)
    gmat = og[:, :, C:2 * C].astype(np.float64)
    new_batch = np.repeat(np.arange(B, dtype=np.int32), C)

    accSR = np.trace(out_adj.astype(np.float64), axis1=1, axis2=2).sum()
    accGq = (gmat ** 2).sum()
    accENT = 0.0
    for c in range(NCORES):
        accENT += res[c]["acc3"].astype(np.float64)[:, 2].sum()

    link_ssq = max(ssqA - 2.0 * accSR + accGq, 0.0)
    link_loss = np.float32(np.sqrt(link_ssq) / float(B * N * N))
    ent_loss = np.float32(-accENT / float(B * N))
    return (x_out, out_adj, new_batch, link_loss, ent_loss)


# revision 22
# speedup vs baseline: 1.0685x; 1.0685x over previous
"""DiffPool (dense_diff_pool + DenseSAGEConv) on 8 Trainium2 NeuronCores.

Sharding: data-parallel over the graph batch dimension B. Each core owns
B/8 = 16 graphs: it densifies its own [16, N, N] adjacency on-device from
per-row CSR index lists (gpsimd local_scatter), runs the SAGE conv +
softmax assignment + pooling bmms per graph entirely out of SBUF, and
emits per-core partials for the scalar link/entropy losses, which the
host combines (equivalent to the all-reduce of two scalars).

Numerics: the adjacency is built in fp16 (integer edge counts, exact);
matmul operands are fp16 with fp32 PSUM accumulation; softmax and loss
reductions run in fp32. The entropy uses the exact softmax identity
ln(softmax) = (logit - max) - ln(sum_exp), so no per-row Ln is needed;
||A||_F^2 is a sum of squared integer edge counts, computed on the host
from the same CSR counts the device scatters.

Host-side work is limited to index arithmetic / layout prep: intra-graph
node positions, per-(graph,node) CSR lists with duplicate-edge counts,
transposing x, and final scalar-loss assembly.
"""

import sys

for _p in ("/opt/trn_rl_repo",):
    if _p not in sys.path:
        sys.path.append(_p)

import numpy as np
import ml_dtypes

import concourse.bass as bass
import concourse.bacc as bacc
import concourse.mybir as mybir
import concourse.tile as tile
from concourse.masks import make_identity
from concourse.bass_utils import run_bass_kernel_spmd

F32 = mybir.dt.float32
F16 = mybir.dt.float16
I16 = mybir.dt.int16
AF = mybir.ActivationFunctionType
ALU = mybir.AluOpType
AX = mybir.AxisListType

B = 128      # graphs in batch
N = 512      # max nodes per graph
D = 128      # node features
C = 64       # clusters
P = 128      # SBUF partitions
NCORES = 8
G = B // NCORES          # graphs per core
TPG = N // P             # 128-row tiles per graph
NT = G * TPG             # v-tiles per core
NT2 = NT // 2            # packed (2-tile) scatter calls per core
EPS = 1e-15
YW = 2 * C + 1           # per-nb y layout: [ones(1) | rel(64) | root(64)]

_KERNEL_CACHE = {}


def _build(NI2, has_bias):
    nc = bacc.Bacc("TRN2", target_bir_lowering=False, debug=False, num_devices=NCORES)

    lsidx = nc.declare_dram_parameter("lsidx", [NT2, P, NI2], I16, isOutput=False)
    lsval = nc.declare_dram_parameter("lsval", [NT2, P, NI2], F16, isOutput=False)
    xt_in = nc.declare_dram_parameter("xt", [G, D, N], F16, isOutput=False)
    xr_in = nc.declare_dram_parameter("xr", [G, N, D], F16, isOutput=False)
    wb_in = nc.declare_dram_parameter("wb", [D + 1, 2 * C], F16, isOutput=False)
    mk_in = nc.declare_dram_parameter("mask", [NT, P], F32, isOutput=False)

    oxt_out = nc.declare_dram_parameter("oxt", [D, G * C], F32, isOutput=True)
    oadj_out = nc.declare_dram_parameter("oadj", [C, G * 2 * C], F32, isOutput=True)
    acc3_out = nc.declare_dram_parameter("acc3", [P, 4], F32, isOutput=True)

    with tile.TileContext(nc) as tc:
        with (
            tc.tile_pool(name="const", bufs=1) as cp,
            tc.tile_pool(name="work", bufs=2) as wp,
            tc.tile_pool(name="work3", bufs=3) as wp3,
            tc.tile_pool(name="work4", bufs=4) as wp4,
            tc.tile_pool(name="ps1", bufs=1, space="PSUM") as pp1,
            tc.tile_pool(name="ps2", bufs=2, space="PSUM") as pp2,
            tc.tile_pool(name="ps3", bufs=3, space="PSUM") as pp3,
        ):
            # ---------- preload ----------
            idx_sb = cp.tile([P, NT2 * NI2], I16)
            nc.sync.dma_start(out=idx_sb[:], in_=lsidx.ap().rearrange("t p i -> p t i"))
            val_sb = cp.tile([P, NT2 * NI2], F16)
            nc.sync.dma_start(out=val_sb[:], in_=lsval.ap().rearrange("t p i -> p t i"))
            wb_sb = cp.tile([P, 2 * C], F16)
            nc.sync.dma_start(out=wb_sb[:], in_=wb_in.ap()[0:D])
            bias_sb = cp.tile([1, 2 * C], F16)
            nc.sync.dma_start(out=bias_sb[:], in_=wb_in.ap()[D:D + 1])
            xt_sb = cp.tile([P, G * N], F16)
            nc.sync.dma_start(out=xt_sb[:], in_=xt_in.ap().rearrange("g d n -> d g n"))
            mask_sb = cp.tile([P, NT], F32)
            nc.sync.dma_start(out=mask_sb[:], in_=mk_in.ap().rearrange("t p -> p t"))
            xr_sb = cp.tile([P, NT * D], F16)
            nc.sync.dma_start(
                out=xr_sb[:], in_=xr_in.ap().rearrange("g (t p) d -> p (g t) d", p=P)
            )

            ones_k1 = cp.tile([1, P], F16)
            nc.vector.memset(ones_k1[:], 1.0)
            warm_sb = cp.tile([P, N], F16)
            nc.vector.memset(warm_sb[:], 0.0)
            for _w in range(40):
                warm_ps = pp3.tile([P, N], F32, tag="r_ps", name=f"warm_{_w}")
                nc.tensor.matmul(out=warm_ps[:], lhsT=warm_sb[:, 0:P],
                                 rhs=warm_sb[:], start=True, stop=True)
            acc3 = cp.tile([P, 4], F32)
            nc.vector.memset(acc3[:], 0.0)
            oxt_sb = cp.tile([D, G * C], F32)
            oadj_sb = cp.tile([C, G * 2 * C], F32)

            # per-(graph, ub) column stats, one column per u-tile
            NM_all = cp.tile([P, NT], F32)   # negated row max of logits
            RS_all = cp.tile([P, NT], F32)   # sum exp
            RI_all = cp.tile([P, NT], F32)   # 1 / sum exp
            ES_all = cp.tile([P, NT], F32)   # sum_c e * slog
            fin_a = cp.tile([P, NT], F32)
            fin_b = cp.tile([P, NT], F32)

            for g in range(G):
                g4 = g * TPG
                # ---------- densify adjacency (adjT, v-partitioned, packed) ----------
                adjp = [wp4.tile([P, 2 * N], F16, tag=f"adjp{h}", name=f"adjp{h}_{g}")
                        for h in range(2)]
                for h in range(2):
                    t2 = g * 2 + h
                    nc.gpsimd.local_scatter(
                        out_ap=adjp[h][:],
                        data_ap=val_sb[:, t2 * NI2:(t2 + 1) * NI2],
                        idxs_ap=idx_sb[:, t2 * NI2:(t2 + 1) * NI2],
                        channels=P, num_elems=2 * N, num_idxs=NI2,
                    )

                def adj_lhsT(vb, ub):
                    off = (vb % 2) * N + ub * P
                    return adjp[vb // 2][:, off:off + P]

                # ---------- y = [1 | x@W_rel^T | x@W_root^T + b_root] ----------
                y_sb = wp3.tile([P, TPG * YW], F16, tag="y")
                nc.vector.memset(
                    y_sb[:].rearrange("p (n w) -> p n w", w=YW)[:, :, 0:1], 1.0)
                y_ps = pp1.tile([P, TPG * 2 * C], F32, tag="y_ps")
                for nb in range(TPG):
                    yo = nb * 2 * C
                    if has_bias:
                        nc.tensor.matmul(out=y_ps[:, yo:yo + 2 * C], lhsT=ones_k1[:],
                                         rhs=bias_sb[:], start=True, stop=False)
                    nc.tensor.matmul(out=y_ps[:, yo:yo + 2 * C],
                                     lhsT=xt_sb[:, g * N + nb * P: g * N + (nb + 1) * P],
                                     rhs=wb_sb[:], start=not has_bias, stop=True)
                nc.any.tensor_copy(
                    out=y_sb[:].rearrange("p (n w) -> p n w", w=YW)[:, :, 1:YW],
                    in_=y_ps[:].rearrange("p (n w) -> p n w", w=2 * C))

                # ---------- z | deg (transposed wide form), logits ----------
                slog_all = wp3.tile([P, TPG * C], F32, tag="slog")
                e_all = wp3.tile([P, TPG * C], F32, tag="e_all")
                recip = wp.tile([P, TPG], F32, tag="recip")
                z_ps = pp2.tile([P, TPG * (C + 1)], F32, tag="z_ps")
                for ub in range(TPG):
                    zo = ub * (C + 1)
                    for vb in range(TPG):
                        nc.tensor.matmul(out=z_ps[:, zo:zo + C + 1],
                                         lhsT=adj_lhsT(vb, ub),
                                         rhs=y_sb[:, vb * YW: vb * YW + C + 1],
                                         start=(vb == 0), stop=(vb == TPG - 1))
                nc.vector.tensor_scalar_max(
                    recip[:, 0:TPG],
                    z_ps[:].rearrange("p (u w) -> p u w", w=C + 1)[:, :, 0:1], 1.0)
                nc.vector.reciprocal(recip[:, 0:TPG], recip[:, 0:TPG])
                for ub in range(TPG):
                    zo = ub * (C + 1)
                    # slog = z * (1/deg) + y_root
                    nc.vector.scalar_tensor_tensor(
                        out=slog_all[:, ub * C:(ub + 1) * C],
                        in0=z_ps[:, zo + 1:zo + C + 1], scalar=recip[:, ub:ub + 1],
                        in1=y_sb[:, ub * YW + C + 1:(ub + 1) * YW],
                        op0=ALU.mult, op1=ALU.add)

                # ---------- softmax ----------
                nc.vector.tensor_reduce(
                    out=NM_all[:, g4:g4 + TPG],
                    in_=slog_all[:].rearrange("p (u c) -> p u c", c=C),
                    axis=AX.X, op=ALU.max, negate=True)
                rs = [wp3.tile([P, 2 * C], F16, tag=f"rs{u}", name=f"rs{u}_{g}")
                      for u in range(TPG)]
                for ub in range(TPG):
                    nc.scalar.activation(
                        out=e_all[:, ub * C:(ub + 1) * C],
                        in_=slog_all[:, ub * C:(ub + 1) * C], func=AF.Exp,
                        bias=NM_all[:, g4 + ub:g4 + ub + 1], scale=1.0,
                        accum_out=RS_all[:, g4 + ub:g4 + ub + 1])
                nc.vector.reciprocal(RI_all[:, g4:g4 + TPG], RS_all[:, g4:g4 + TPG])
                for ub in range(TPG):
                    nc.any.tensor_scalar(
                        out=rs[ub][:, C:2 * C], in0=e_all[:, ub * C:(ub + 1) * C],
                        scalar1=RI_all[:, g4 + ub:g4 + ub + 1],
                        scalar2=mask_sb[:, g4 + ub:g4 + ub + 1],
                        op0=ALU.mult, op1=ALU.mult)

                # ---------- R = A @ sm ----------
                for ub in range(TPG):
                    r_ps = pp3.tile([P, C], F32, tag="r_ps")
                    for vb in range(TPG):
                        nc.tensor.matmul(out=r_ps[:], lhsT=adj_lhsT(vb, ub),
                                         rhs=rs[vb][:, C:2 * C],
                                         start=(vb == 0), stop=(vb == TPG - 1))
                    nc.any.tensor_copy(out=rs[ub][:, 0:C], in_=r_ps[:])

                # ---------- out_adj | G  and  out_x^T ----------
                og_ps_t = pp1.tile([C, 2 * C], F32, tag="og_ps")
                ox_ps_t = pp1.tile([D, C], F32, tag="ox_ps")
                og_ps = og_ps_t[:]
                ox_ps = ox_ps_t[:]
                for ub in range(TPG):
                    nc.tensor.matmul(out=og_ps, lhsT=rs[ub][:, C:2 * C],
                                     rhs=rs[ub][:], start=(ub == 0), stop=(ub == TPG - 1))
                    nc.tensor.matmul(out=ox_ps,
                                     lhsT=xr_sb[:, (g4 + ub) * D:(g4 + ub + 1) * D],
                                     rhs=rs[ub][:, C:2 * C],
                                     start=(ub == 0), stop=(ub == TPG - 1))
                nc.any.tensor_copy(out=oadj_sb[:, g * 2 * C:(g + 1) * 2 * C], in_=og_ps)
                nc.any.tensor_copy(out=oxt_sb[:, g * C:(g + 1) * C], in_=ox_ps)

                # ---------- entropy pieces: sum_c e * slog ----------
                eslog = wp3.tile([P, TPG * C], F32, tag="eslog")
                nc.vector.tensor_mul(out=eslog[:], in0=e_all[:], in1=slog_all[:])
                nc.vector.tensor_reduce(
                    out=ES_all[:, g4:g4 + TPG],
                    in_=eslog[:].rearrange("p (u c) -> p u c", c=C),
                    axis=AX.X, op=ALU.add)


            # ---------- entropy: sum_row mask * (ES*RI - (ln(RS) - NM)) ----------
            nc.scalar.activation(out=fin_a[:], in_=RS_all[:], func=AF.Ln)
            nc.vector.tensor_tensor(out=fin_a[:], in0=fin_a[:], in1=NM_all[:],
                                    op=ALU.subtract)
            nc.vector.tensor_mul(out=fin_b[:], in0=ES_all[:], in1=RI_all[:])
            nc.vector.tensor_tensor(out=fin_b[:], in0=fin_b[:], in1=fin_a[:],
                                    op=ALU.subtract)
            nc.vector.tensor_mul(out=fin_b[:], in0=fin_b[:], in1=mask_sb[:])
            nc.vector.tensor_reduce(out=acc3[:, 2:3], in_=fin_b[:], axis=AX.X,
                                    op=ALU.add)

            nc.sync.dma_start(out=oxt_out.ap(), in_=oxt_sb[:])
            nc.sync.dma_start(out=oadj_out.ap(), in_=oadj_sb[:])
            nc.sync.dma_start(out=acc3_out.ap(), in_=acc3[:])

    nc.finalize()
    return nc


def _get_nc(NI2, has_bias):
    key = (NI2, has_bias)
    if key not in _KERNEL_CACHE:
        _KERNEL_CACHE[key] = _build(NI2, has_bias)
    return _KERNEL_CACHE[key]


def _prep_inputs(x, edge_index, batch, W_rel, W_root, b_root):
    total = x.shape[0]
    counts = np.bincount(batch, minlength=B)
    ptr = np.zeros(B + 1, np.int64)
    np.cumsum(counts, out=ptr[1:])
    pos = np.arange(total, dtype=np.int64) - ptr[batch]

    if total == B * N and np.all(counts == N):
        dense_x = np.ascontiguousarray(x.reshape(B, N, D))
        mask = np.ones((B, N), np.float32)
    else:
        dense_x = np.zeros((B, N, D), np.float32)
        mask = np.zeros((B, N), np.float32)
        sel = pos < N
        dense_x[batch[sel], pos[sel]] = x[sel]
        mask[batch[sel], pos[sel]] = 1.0

    src, dst = edge_index[0].astype(np.int64), edge_index[1].astype(np.int64)
    b_e = batch[src].astype(np.int64)
    us, vs = pos[src], pos[dst]
    ok = (us < N) & (vs < N)
    key = ((b_e * N + vs) * N + us)[ok]  # adjT cell: row = g*N + v, col = u
    key.sort()
    if key.size:
        first = np.empty(key.size, bool)
        first[0] = True
        np.not_equal(key[1:], key[:-1], out=first[1:])
        ucell = key[first]
        starts = np.flatnonzero(first)
        cnt = np.diff(np.append(starts, key.size))
    else:
        ucell = key
        cnt = np.zeros(0, np.int64)
    ssqA = float((cnt.astype(np.float64) ** 2).sum())  # ||A||_F^2, exact

    rows = ucell // N                       # global g*N + v
    colu = ucell % N                        # u
    row_start = np.searchsorted(rows, np.arange(B * N))
    row_start_ext = np.append(row_start, ucell.size)
    rowcnt = np.diff(row_start_ext)         # unique-u count per adjT row
    pir = np.arange(ucell.size, dtype=np.int64) - row_start[rows]

    # packed scatter tiles: tile t2 = row // 256 covers 256 consecutive rows;
    # partition p = row % 128; rows with (row % 256) >= 128 go to cols +512.
    is_hi = (rows % 256) >= 128
    slot = pir + np.where(is_hi, rowcnt[np.maximum(rows - 128, 0)], 0)
    t2 = rows // 256
    pp = rows % 128
    colp = colu + np.where(is_hi, N, 0)
    max_slot = int(slot.max()) + 1 if ucell.size else 2
    NI2 = max(80, 2 * ((max_slot + 1) // 2))

    idx_np = np.full((B * 2, P, NI2), -1, np.int16)
    val_np = np.zeros((B * 2, P, NI2), np.float16)
    idx_np[t2, pp, slot] = colp.astype(np.int16)
    val_np[t2, pp, slot] = cnt

    wb = np.zeros((D + 1, 2 * C), np.float32)
    wb[0:D, 0:C] = W_rel.T
    wb[0:D, C:2 * C] = W_root.T
    wb[D, C:2 * C] = b_root

    xt = np.ascontiguousarray(
        dense_x.reshape(NCORES, G, N, D).transpose(0, 1, 3, 2)).astype(np.float16)
    xr = dense_x.reshape(NCORES, G, N, D).astype(np.float16)
    idx4 = idx_np.reshape(NCORES, NT2, P, NI2)
    val4 = val_np.reshape(NCORES, NT2, P, NI2)
    mk = mask.reshape(NCORES, NT, P)
    wb16 = wb.astype(np.float16)

    in_maps = []
    for c in range(NCORES):
        in_maps.append({
            "lsidx": idx4[c], "lsval": val4[c],
            "xt": xt[c], "xr": np.ascontiguousarray(xr[c]),
            "wb": wb16, "mask": np.ascontiguousarray(mk[c]),
        })
    return in_maps, NI2, ssqA


def kernel(x, edge_index, batch, W_rel, W_root, b_root):
    x = np.asarray(x, np.float32)
    edge_index = np.asarray(edge_index, np.int32)
    batch = np.asarray(batch, np.int32)
    W_rel = np.asarray(W_rel, np.float32)
    W_root = np.asarray(W_root, np.float32)
    b_root = np.asarray(b_root, np.float32)

    in_maps, NI2, ssqA = _prep_inputs(x, edge_index, batch, W_rel, W_root, b_root)
    nc = _get_nc(NI2, bool(np.any(b_root != 0.0)))
    res = run_bass_kernel_spmd(nc, in_maps, list(range(NCORES))).results

    oxt = np.stack([res[c]["oxt"] for c in range(NCORES)])    # [8, D, G*C]
    oadj = np.stack([res[c]["oadj"] for c in range(NCORES)])  # [8, C, G*2C]
    x_out = (oxt.reshape(NCORES, D, G, C)
             .transpose(0, 2, 3, 1).reshape(B * C, D).astype(np.float32))
    og = oadj.reshape(NCORES, C, G, 2 * C).transpose(0, 2, 1, 3).reshape(B, C, 2 * C)
    out_adj = np.ascontiguousarray(og[:, :, 0:C]).astype(np.float32)
    gmat = og[:, :, C:2 * C].astype(np.float64)
    new_batch = np.repeat(np.arange(B, dtype=np.int32), C)

    accSR = np.trace(out_adj.astype(np.float64), axis1=1, axis2=2).sum()
    accGq = (gmat ** 2).sum()
    accENT = 0.0
    for c in range(NCORES):
        accENT += res[c]["acc3"].astype(np.float64)[:, 2].sum()

    link_ssq = max(ssqA - 2.0 * accSR + accGq, 0.0)
    link_loss = np.float32(np.sqrt(link_ssq) / float(B * N * N))
    ent_loss = np.float32(-accENT / float(B * N))
    return (x_out, out_adj, new_batch, link_loss, ent_loss)
